# revision 27
# baseline (speedup 1.0000x reference)
"""Trainium2 Bass kernel for DeformableSubspaceModulatedConv2d.

Contract: kernel(**inputs) takes FULL unsharded inputs (as produced by
setup_inputs) and returns the FULL output [16, 512, 64, 64] f32.

Strategy (data-parallel over batch, 2 samples per core on 8 cores),
Winograd F(2x2, 3x3):
  host layout prep: pad+column-deinterleave x (bf16), relayout
    weight/basis; 8x8 basis Gram matrix -> per-sample delta-norm scalar
    k_b = shift / max(||sum_j c_j bv_j||, 1e-12).
  P0: s[i,b] = style @ mod_w.T + mod_b                     (PE)
  W-phase per (ib, s): psum = (s_i/k)*wt + sum_j (c_j s_i)*bv_j
    via fp8 DoubleRow pair-matmuls (basis) + bf16 identity matmul (wt);
    ACT evac (scale k) -> W1s bf16 = s*(wt + k*delta_unnorm).
    DVE Winograd weight transform (integer G) -> U[i, uv, o] bf16.
    demod accum: ACT Square(W1s) + PE ones-contraction -> drow.
  C-phase per sample, tile-chunks of 256 (8 tile-rows x 32 tile-cols):
    DVE input transform (col-stage on deinterleaved cols, row-stage) ->
    V bf16; PE matmuls (16 uv x 4 ob x 4 ib, 256-col) accumulating over
    i; ACT evac psum->Mst bf16 with per-class Winograd scale * demod;
    DVE inverse transform (A^T M A) -> bf16 out tiles; DMA out.
  host: reassemble [y,x,tile] layout to [o,h,w], cast f32.
"""

import sys

sys.path.insert(0, "/opt/trn_rl_repo")

import numpy as np
import ml_dtypes
from contextlib import ExitStack

import concourse.bass as bass
import concourse.tile as tile
from concourse import bacc, bass_utils, mybir

F32 = mybir.dt.float32
BF16 = mybir.dt.bfloat16
FP8 = mybir.dt.float8e4
AF = mybir.ActivationFunctionType
ALU = mybir.AluOpType
DR = mybir.MatmulPerfMode.DoubleRow

B, CIN, COUT, K, H, W = 16, 512, 512, 3, 64, 64
STYLE_DIM, BASIS, DIRS = 512, 8, 8
NCORES = 8
BLOC = B // NCORES  # 2 samples per core
NIB = CIN // 128  # 4 i blocks
NOB = COUT // 128  # 4 o blocks
KK = K * K  # 9
NCH = 4  # tile chunks per sample (8 tile-rows each)
NTR = 8  # tile-rows per chunk
NTC = 32  # tile-cols
NPAIR = BASIS // 2
SCALE = 1.0 / np.sqrt(CIN * K * K)
# v (and l) natural index -> class-ordered slot: v in {0,3} -> {0,1}, {1,2} -> {2,3}
VSLOT = {0: 0, 3: 1, 1: 2, 2: 3}

_NC_CACHE = {}
_RUN_KWARGS = {}
_LAST_RESULT = {}


def _build():
    nc = bacc.Bacc("TRN2", target_bir_lowering=False, debug=False)

    # ---- DRAM tensors ----
    xde_d = nc.dram_tensor("xde", [BLOC, NCH, NIB, 128, 18, 66], BF16, kind="ExternalInput")
    styleT_d = nc.dram_tensor("styleT", [STYLE_DIM, BLOC], F32, kind="ExternalInput")
    mod_wT_d = nc.dram_tensor("mod_wT", [STYLE_DIM, CIN], F32, kind="ExternalInput")
    modb_d = nc.dram_tensor("mod_b_t", [128, NIB], F32, kind="ExternalInput")
    wt_d = nc.dram_tensor("wt", [NIB, 128, KK, COUT], BF16, kind="ExternalInput")
    bvp_d = nc.dram_tensor("bvp", [NPAIR, NIB, 128, KK, 2, COUT], FP8, kind="ExternalInput")
    cbc_d = nc.dram_tensor("c_bcast", [128, BLOC * BASIS], F32, kind="ExternalInput")
    kinv_d = nc.dram_tensor("kinv_bcast", [128, BLOC], F32, kind="ExternalInput")
    kt_d = nc.dram_tensor("kt_bcast", [128, BLOC], F32, kind="ExternalInput")
    ident_d = nc.dram_tensor("identity_bf", [128, 128], BF16, kind="ExternalInput")
    ident8_d = nc.dram_tensor("identity_f8", [128, 128], FP8, kind="ExternalInput")
    ones_d = nc.dram_tensor("ones_col", [128, 1], F32, kind="ExternalInput")
    out_d = nc.dram_tensor("out", [BLOC, NOB, NCH, 128, 2, 2, NTR, NTC], BF16, kind="ExternalOutput")
    w1s_d = nc.dram_tensor("w1s_scratch", [NIB, 128, KK, COUT], BF16, kind="Internal")

    with tile.TileContext(nc) as tc, ExitStack() as top:
        persist = top.enter_context(tc.tile_pool(name="persist", bufs=1))

        ident_t = persist.tile([128, 128], BF16, tag="ident")
        nc.sync.dma_start(ident_t[:], ident_d.ap())
        ident8_t = persist.tile([128, 128], FP8, tag="ident8")
        nc.sync.dma_start(ident8_t[:], ident8_d.ap())
        cbc_t = persist.tile([128, BLOC * BASIS], F32, tag="cbc")
        nc.sync.dma_start(cbc_t[:], cbc_d.ap())
        kinv_t = persist.tile([128, BLOC], F32, tag="kinv")
        nc.sync.dma_start(kinv_t[:], kinv_d.ap())
        kt_t = persist.tile([128, BLOC], F32, tag="kt")
        nc.sync.dma_start(kt_t[:], kt_d.ap())
        modb_t = persist.tile([128, NIB], F32, tag="modb")
        nc.sync.dma_start(modb_t[:], modb_d.ap())
        ones_t = persist.tile([128, 1], F32, tag="ones")
        nc.sync.dma_start(ones_t[:], ones_d.ap())
        ones_bf = persist.tile([128, 1], BF16, tag="onesbf")
        nc.vector.tensor_scalar_mul(ones_bf[:], ones_t[:], 1.0)
        s_sb = persist.tile([128, NIB, BLOC], F32, tag="s_sb")
        sk_sb = persist.tile([128, NIB, BLOC], F32, tag="sk_sb")

        # ---- P0: style modulation s[i, b] ----
        with ExitStack() as p0:
            mw_pool = p0.enter_context(tc.tile_pool(name="mw", bufs=NIB))
            st_pool = p0.enter_context(tc.tile_pool(name="st", bufs=1))
            p0_psum = p0.enter_context(tc.tile_pool(name="p0ps", bufs=1, space="PSUM"))
            stT = st_pool.tile([128, NIB, BLOC], F32, tag="styleT")
            nc.sync.dma_start(stT[:], styleT_d.ap().rearrange("(db p) b -> p db b", p=128))
            mw_t = []
            for db in range(NIB):
                t = mw_pool.tile([128, CIN], F32, tag="mw")
                nc.sync.dma_start(t[:], mod_wT_d.ap()[db * 128 : (db + 1) * 128, :])
                mw_t.append(t)
            for ib in range(NIB):
                ps = p0_psum.tile([128, BLOC], F32, tag="ps_s")
                for db in range(NIB):
                    nc.tensor.matmul(
                        ps[:],
                        mw_t[db][:, ib * 128 : (ib + 1) * 128],
                        stT[:, db, :],
                        start=(db == 0),
                        stop=(db == NIB - 1),
                    )
                for s in range(BLOC):
                    nc.vector.tensor_add(
                        s_sb[:, ib, s : s + 1], ps[:, s : s + 1], modb_t[:, ib : ib + 1]
                    )
                    # sk = s_i / k_b
                    nc.vector.tensor_mul(
                        sk_sb[:, ib, s : s + 1],
                        s_sb[:, ib, s : s + 1],
                        kinv_t[:, s : s + 1],
                    )

        # ---- main pools ----
        ci_pool = top.enter_context(tc.tile_pool(name="ci", bufs=12))
        ds_pool = top.enter_context(tc.tile_pool(name="ds", bufs=1))
        bvp_pool = top.enter_context(tc.tile_pool(name="bvp", bufs=5))
        wtc_pool = top.enter_context(tc.tile_pool(name="wtc", bufs=1))
        wts_pool = top.enter_context(tc.tile_pool(name="wts", bufs=1))
        w1s_pool = top.enter_context(tc.tile_pool(name="w1s", bufs=1))
        g_pool = top.enter_context(tc.tile_pool(name="g", bufs=1))
        u_pool = top.enter_context(tc.tile_pool(name="u", bufs=1))
        sq_pool = top.enter_context(tc.tile_pool(name="sq", bufs=1))
        dem_pool = top.enter_context(tc.tile_pool(name="dem", bufs=4))
        drow_pool = top.enter_context(tc.tile_pool(name="drow", bufs=1))
        xde_pool = top.enter_context(tc.tile_pool(name="xde", bufs=1))
        f_pool = top.enter_context(tc.tile_pool(name="f", bufs=1))
        v_pool = top.enter_context(tc.tile_pool(name="v", bufs=16))
        mst_pool = top.enter_context(tc.tile_pool(name="mst", bufs=4))
        sy_pool = top.enter_context(tc.tile_pool(name="sy", bufs=1))
        it_pool = top.enter_context(tc.tile_pool(name="it", bufs=1))
        outt_pool = top.enter_context(tc.tile_pool(name="outt", bufs=1))
        pd_psum = top.enter_context(tc.tile_pool(name="pd", bufs=2, space="PSUM"))
        pc_psum = top.enter_context(tc.tile_pool(name="pc", bufs=2, space="PSUM"))
        psd_psum = top.enter_context(tc.tile_pool(name="psd", bufs=2, space="PSUM"))

        # U tiles: one per ib, rewritten per sample
        u_t = [
            u_pool.tile([128, 16, COUT], BF16, tag=f"u{ib}", name=f"u{ib}")
            for ib in range(NIB)
        ]

        def make_ci(ib, s):
            """ci pair tiles [128, 2, 128] fp8 = diag(s) * c_j for this (ib, s)."""
            ds = ds_pool.tile([128, 128], FP8, tag="ds")
            nc.vector.tensor_scalar_mul(ds[:], ident8_t[:], s_sb[:, ib, s : s + 1])
            cis = []
            for p in range(NPAIR):
                t = ci_pool.tile([128, 2, 128], FP8, tag="ci")
                for half in range(2):
                    j = 2 * p + half
                    nc.vector.tensor_scalar_mul(
                        t[:, half, :], ds[:], cbc_t[:, s * BASIS + j : s * BASIS + j + 1]
                    )
                cis.append(t)
            return cis

        def weight_transform(w1s, ib):
            """DVE Winograd weight transform W1s [128,9,512] -> u_t[ib] [128,16,512]
            with the su*sv class scales folded in (su = [1,.5,.5,1])."""
            U = u_t[ib]
            tmpg = g_pool.tile([128, 3, COUT], BF16, tag="tmpg")
            g1 = g_pool.tile([128, 3, COUT], BF16, tag="g1")
            g2 = g_pool.tile([128, 3, COUT], BF16, tag="g2")
            nc.vector.tensor_add(tmpg[:], w1s[:, 0:3, :], w1s[:, 6:9, :])
            nc.vector.tensor_add(g1[:], tmpg[:], w1s[:, 3:6, :])
            nc.vector.tensor_sub(g2[:], tmpg[:], w1s[:, 3:6, :])
            # su scale for rows u in {1,2}
            nc.vector.tensor_scalar_mul(g1[:], g1[:], 0.5)
            nc.vector.tensor_scalar_mul(g2[:], g2[:], 0.5)
            gu = [w1s[:, 0:3, :], g1[:], g2[:], w1s[:, 6:9, :]]
            for u in range(4):
                gs = gu[u]
                base = u * 4
                nc.vector.tensor_copy(U[:, base + 0, :], gs[:, 0, :])
                nc.vector.tensor_copy(U[:, base + 1, :], gs[:, 2, :])
                tmpc = g_pool.tile([128, COUT], BF16, tag="tmpc")
                nc.vector.tensor_add(tmpc[:], gs[:, 0, :], gs[:, 2, :])
                th = g_pool.tile([128, COUT], BF16, tag="th")
                nc.vector.tensor_scalar_mul(th[:], tmpc[:], 0.5)
                gh = g_pool.tile([128, COUT], BF16, tag="gh")
                nc.vector.tensor_scalar_mul(gh[:], gs[:, 1, :], 0.5)
                nc.vector.tensor_add(U[:, base + 2, :], th[:], gh[:])
                nc.vector.tensor_sub(U[:, base + 3, :], th[:], gh[:])

        def demod_accum(w1s, ib, dsum):
            """ACT square + PE ones-contraction; per-(ib,ob) closed psum group,
            partials accumulated into SBUF row dsum [1, NOB*128]."""
            for ob in range(NOB):
                sq = sq_pool.tile([128, KK, 128], BF16, tag="sq")
                nc.scalar.activation(
                    sq[:], w1s[:, :, ob * 128 : (ob + 1) * 128], AF.Square
                )
                pp = psd_psum.tile([1, 128], F32, tag="psd", name="pp")
                for kk in range(KK):
                    nc.tensor.matmul(
                        pp[:],
                        ones_bf[:],
                        sq[:, kk, :],
                        start=(kk == 0),
                        stop=(kk == KK - 1),
                    )
                dslice = dsum[0:1, ob * 128 : (ob + 1) * 128]
                if ib == 0:
                    nc.vector.tensor_copy(dslice, pp[:])
                else:
                    nc.vector.tensor_add(dslice, dslice, pp[:])

        def demod_finish(s, dsum):
            """drow per ob -> dem_cls [128, 3] tiles (dem*{1,.5,.25})."""
            dem_cls = []
            for ob in range(NOB):
                vv = drow_pool.tile([1, 128], F32, tag="vv")
                nc.vector.tensor_scalar(
                    vv[:], dsum[0:1, ob * 128 : (ob + 1) * 128], SCALE * SCALE, 1e-8,
                    op0=ALU.mult, op1=ALU.add,
                )
                rr = drow_pool.tile([1, 128], F32, tag="rr")
                nc.vector.reciprocal(rr[:], vv[:])
                hh = drow_pool.tile([1, 128], F32, tag="hh")
                nc.scalar.sqrt(hh[:], rr[:])
                drow = drow_pool.tile([1, 128], F32, tag="drow")
                nc.vector.tensor_scalar_mul(drow[:], hh[:], SCALE)
                pst = pd_psum.tile([128, 1], F32, tag="pd", name="pst")
                nc.tensor.matmul(pst[:], drow[:], ones_t[0:1, 0:1])
                dc = dem_pool.tile([128, 1], F32, tag="demc")
                nc.vector.tensor_copy(dc[:], pst[:])
                dem_cls.append(dc)
            return dem_cls

        def delta_w1s_both(ib, cis2):
            """Compute W1s bf16 [128, KK, COUT] for both samples of (ib),
            streaming each basis kk-group once."""
            w1s2 = [
                w1s_pool.tile([128, KK, COUT], BF16, tag=f"w1s{s}", name=f"w1s{s}")
                for s in range(BLOC)
            ]
            for g in range(3):
                bvg = []
                for p in range(NPAIR):
                    t = bvp_pool.tile([128, 3, 2, COUT], FP8, tag="bvp", name=f"bvg{p}")
                    nc.sync.dma_start(t[:], bvp_d.ap()[p, ib, :, 3 * g : 3 * g + 3, :, :])
                    bvg.append(t)
                wtcg = wtc_pool.tile([128, 3, COUT], BF16, tag="wtc")
                nc.sync.dma_start(wtcg[:], wt_d.ap()[ib, :, 3 * g : 3 * g + 3, :])
                for s in range(BLOC):
                    for kl in range(3):
                        kk = 3 * g + kl
                        wts = wts_pool.tile([128, COUT], BF16, tag="wts")
                        nc.scalar.activation(
                            wts[:], wtcg[:, kl, :], AF.Copy,
                            scale=sk_sb[:, ib, s : s + 1],
                        )
                        ps = pd_psum.tile([128, COUT], F32, tag="pd")
                        for p in range(NPAIR):
                            nc.tensor.matmul(
                                ps[:],
                                cis2[s][p][:],
                                bvg[p][:, kl, :, :],
                                start=(p == 0),
                                stop=False,
                                perf_mode=DR,
                            )
                        nc.tensor.matmul(ps[:], ident_t[:], wts[:], start=False, stop=True)
                        nc.scalar.activation(
                            w1s2[s][:, kk, :], ps[:], AF.Copy, scale=kt_t[:, s : s + 1]
                        )
            return w1s2

        # ---- W-phase: sample 0 transforms + sample 1 staged to DRAM ----
        psd_s0 = persist.tile([1, NOB * 128], F32, tag="dsum0", name="dsum0")
        for ib in range(NIB):
            cis2 = [make_ci(ib, s) for s in range(BLOC)]
            w1s2 = delta_w1s_both(ib, cis2)
            # sample 0: full pipeline to U; sample 1: stash to DRAM
            weight_transform(w1s2[0], ib)
            demod_accum(w1s2[0], ib, psd_s0)
            nc.sync.dma_start(w1s_d.ap()[ib], w1s2[1][:])
        dem_cls_s = [demod_finish(0, psd_s0), None]

        # ---- C-phase ----
        def conv_sample(s, dem_cls):
            for ch in range(NCH):
                v_t = {}
                for ib in range(NIB):
                    xs = xde_pool.tile([128, 18, 66], BF16, tag="xde")
                    nc.sync.dma_start(xs[:], xde_d.ap()[s, ch, ib])
                    xv = xs[:].rearrange("p r (two c) -> p r two c", two=2)
                    f4 = f_pool.tile([128, 18, 4, NTC], BF16, tag="f4")
                    E0 = xv[:, :, 0, 0:32]
                    E1 = xv[:, :, 0, 1:33]
                    O0 = xv[:, :, 1, 0:32]
                    O1 = xv[:, :, 1, 1:33]
                    nc.vector.tensor_sub(f4[:, :, 0, :], E0, E1)
                    nc.vector.tensor_add(f4[:, :, 1, :], O0, E1)
                    nc.vector.tensor_sub(f4[:, :, 2, :], E1, O0)
                    nc.vector.tensor_sub(f4[:, :, 3, :], O0, O1)
                    fr = f4[:].rearrange("p (rp two) l c -> p rp two l c", two=2)
                    r1 = fr[:, 0:8, 1, :, :]
                    r2 = fr[:, 1:9, 0, :, :]
                    for k in range(4):
                        vt = v_pool.tile([128, NTR, 4, NTC], BF16, tag="v")
                        if k == 0:
                            nc.vector.tensor_sub(vt[:], fr[:, 0:8, 0, :, :], r2)
                        elif k == 1:
                            nc.vector.tensor_add(vt[:], r1, r2)
                        elif k == 2:
                            nc.vector.tensor_sub(vt[:], r2, r1)
                        else:
                            nc.vector.tensor_sub(vt[:], r1, fr[:, 1:9, 1, :, :])
                        v_t[(ib, k)] = vt
                msts = [
                    mst_pool.tile([128, 4, 4, 256], BF16, tag="mst", name=f"mst{ob}")
                    for ob in range(NOB)
                ]
                for u in range(4):
                    for ob in range(NOB):
                        psq = pc_psum.tile([128, 4, 256], F32, tag="pc")
                        for v in range(4):
                            slot = VSLOT[v]
                            for ib in range(NIB):
                                nc.tensor.matmul(
                                    psq[:, slot, :],
                                    u_t[ib][:, u * 4 + slot, ob * 128 : (ob + 1) * 128],
                                    v_t[(ib, u)][:, :, v, :],
                                    start=(ib == 0),
                                    stop=(ib == NIB - 1),
                                )
                        nc.scalar.activation(
                            msts[ob][:, u, :, :], psq[:], AF.Copy,
                            scale=dem_cls[ob][:],
                        )
                for ob in range(NOB):
                    mst = msts[ob]
                    # inverse transform
                    sy = sy_pool.tile([128, 2, 4, 256], BF16, tag="sy")
                    tmp1 = it_pool.tile([128, 4, 256], BF16, tag="it1")
                    nc.gpsimd.tensor_add(tmp1[:], mst[:, 0, :, :], mst[:, 1, :, :])
                    nc.vector.tensor_add(sy[:, 0, :, :], tmp1[:], mst[:, 2, :, :])
                    tmp2 = it_pool.tile([128, 4, 256], BF16, tag="it2", name="tmp2")
                    nc.gpsimd.tensor_sub(tmp2[:], mst[:, 1, :, :], mst[:, 2, :, :])
                    nc.vector.tensor_sub(sy[:, 1, :, :], tmp2[:], mst[:, 3, :, :])
                    outt = outt_pool.tile([128, 2, 2, NTR, NTC], BF16, tag="outt")
                    l0, l1, l2, l3 = VSLOT[0], VSLOT[1], VSLOT[2], VSLOT[3]
                    ta = it_pool.tile([128, 2, 256], BF16, tag="ita")
                    nc.vector.tensor_add(ta[:], sy[:, :, l1, :], sy[:, :, l2, :])
                    ov0 = outt[:, :, 0, :, :].rearrange("p y r c -> p y (r c)")
                    nc.vector.tensor_add(ov0, ta[:], sy[:, :, l0, :])
                    tb = it_pool.tile([128, 2, 256], BF16, tag="ita", name="tb")
                    nc.vector.tensor_sub(tb[:], sy[:, :, l1, :], sy[:, :, l2, :])
                    ov1 = outt[:, :, 1, :, :].rearrange("p y r c -> p y (r c)")
                    nc.vector.tensor_sub(ov1, tb[:], sy[:, :, l3, :])
                    nc.sync.dma_start(out_d.ap()[s, ob, ch], outt[:])

        conv_sample(0, dem_cls_s[0])

        # ---- sample 1: reload W1s, transform, demod, conv ----
        psd_s1 = persist.tile([1, NOB * 128], F32, tag="dsum1", name="dsum1")
        for ib in range(NIB):
            w1s1 = w1s_pool.tile([128, KK, COUT], BF16, tag="w1s1", name="w1s1r")
            nc.sync.dma_start(w1s1[:], w1s_d.ap()[ib])
            weight_transform(w1s1, ib)
            demod_accum(w1s1, ib, psd_s1)
        dem_cls_s[1] = demod_finish(1, psd_s1)
        conv_sample(1, dem_cls_s[1])

    nc.compile()
    return nc


def _get_nc():
    if "nc" not in _NC_CACHE:
        _NC_CACHE["nc"] = _build()
    return _NC_CACHE["nc"]


def kernel(**inputs):
    x = np.asarray(inputs["x"], dtype=np.float32)
    style = np.asarray(inputs["style"], dtype=np.float32)
    weight = np.asarray(inputs["weight"], dtype=np.float32)
    mod_w = np.asarray(inputs["mod_w"], dtype=np.float32)
    mod_b = np.asarray(inputs["mod_b"], dtype=np.float32)
    bv = np.asarray(inputs["basis_vectors"], dtype=np.float32)
    shifts_coords = np.asarray(inputs["shifts_coords"], dtype=np.float32)
    batch_shifts = np.asarray(inputs["batch_shifts"], dtype=np.float32)
    batch_directions = np.asarray(inputs["batch_directions"])

    # ---- host-side coefficient prep ----
    coefs = shifts_coords[batch_directions].astype(np.float64)  # [B, 8]
    bvf = bv.reshape(BASIS, -1).astype(np.float64)
    G8 = bvf @ bvf.T
    nrm2 = np.einsum("bi,ij,bj->b", coefs, G8, coefs)
    nrm = np.sqrt(np.maximum(nrm2, 0.0))
    k = batch_shifts.astype(np.float64) / np.maximum(nrm, 1e-12)
    k_eff = np.maximum(k, 1e-20)
    kinv = (1.0 / k_eff).astype(np.float32)
    kt = k_eff.astype(np.float32)
    coefs = coefs.astype(np.float32)

    # ---- host-side layout prep (shared across cores) ----
    # weight [1,O,I,3,3] -> [ib, i, kk, o] bf16
    wtl = weight[0].transpose(1, 2, 3, 0).reshape(NIB, 128, KK, COUT)
    wt_bf = np.ascontiguousarray(wtl).astype(ml_dtypes.bfloat16)
    # basis [8,1,O,I,3,3] -> pairs [pair, ib, i, kk, 2, o] fp8
    bvl = bv[:, 0].transpose(0, 2, 3, 4, 1).reshape(BASIS, NIB, 128, KK, COUT)
    bvp = np.ascontiguousarray(
        bvl.reshape(NPAIR, 2, NIB, 128, KK, COUT).transpose(0, 2, 3, 4, 1, 5)
    ).astype(ml_dtypes.float8_e4m3)
    mod_wT = np.ascontiguousarray(mod_w.T)
    mod_b_t = np.ascontiguousarray(mod_b.reshape(NIB, 128).T)
    identity_bf = np.eye(128, dtype=ml_dtypes.bfloat16)
    identity_f8 = np.eye(128, dtype=ml_dtypes.float8_e4m3)
    ones_col = np.ones((128, 1), np.float32)

    # x: pad to 66x66, deinterleave columns, slab by chunk (18 rows, overlap 2)
    xb = x.astype(ml_dtypes.bfloat16)
    xp = np.zeros((B, CIN, H + 2, W + 2), dtype=ml_dtypes.bfloat16)
    xp[:, :, 1 : H + 1, 1 : W + 1] = xb
    # [B, CIN, 66, 2, 33]: col c = 2*cc + parity
    xd = xp.reshape(B, CIN, H + 2, 33, 2).transpose(0, 1, 2, 4, 3)
    # slabs [B, NCH, CIN, 18, 2, 33]
    xs = np.stack([xd[:, :, 16 * ch : 16 * ch + 18] for ch in range(NCH)], axis=1)
    xs = np.ascontiguousarray(xs.reshape(B, NCH, NIB, 128, 18, 66))

    in_maps = []
    for c in range(NCORES):
        sl = slice(c * BLOC, (c + 1) * BLOC)
        in_maps.append(
            {
                "xde": np.ascontiguousarray(xs[sl]),
                "styleT": np.ascontiguousarray(style[sl].T),
                "mod_wT": mod_wT,
                "mod_b_t": mod_b_t,
                "wt": wt_bf,
                "bvp": bvp,
                "c_bcast": np.ascontiguousarray(
                    np.broadcast_to(coefs[sl].reshape(1, -1), (128, BLOC * BASIS))
                ).astype(np.float32),
                "kinv_bcast": np.ascontiguousarray(
                    np.broadcast_to(kinv[sl].reshape(1, -1), (128, BLOC))
                ),
                "kt_bcast": np.ascontiguousarray(
                    np.broadcast_to(kt[sl].reshape(1, -1), (128, BLOC))
                ),
                "identity_bf": identity_bf,
                "identity_f8": identity_f8,
                "ones_col": ones_col,
            }
        )

    nc = _get_nc()
    res = bass_utils.run_bass_kernel_spmd(
        nc, in_maps, core_ids=list(range(NCORES)), **_RUN_KWARGS
    )
    _LAST_RESULT["res"] = res
    # reassemble: out_d [BLOC, NOB, NCH, 128, 2y, 2x, 8tr, 32tc]
    outs = []
    for c in range(NCORES):
        o = np.asarray(res.results[c]["out"]).astype(np.float32)
        # -> [b, ob, o128, ch, tr, y, tc, x] -> [b, 512, 64, 64]
        o = o.transpose(0, 1, 3, 2, 6, 4, 7, 5).reshape(BLOC, COUT, H, W)
        outs.append(o)
    return np.concatenate(outs, axis=0)


# revision 28
# speedup vs baseline: 1.1522x; 1.1522x over previous
"""Trainium2 Bass kernel for DeformableSubspaceModulatedConv2d.

Contract: kernel(**inputs) takes FULL unsharded inputs (as produced by
setup_inputs) and returns the FULL output [16, 512, 64, 64] f32.

Strategy (data-parallel over batch, 2 samples per core on 8 cores),
Winograd F(2x2, 3x3):
  host layout prep: pad+column-deinterleave x (bf16), relayout
    weight/basis; 8x8 basis Gram matrix -> per-sample delta-norm scalar
    k_b = shift / max(||sum_j c_j bv_j||, 1e-12).
  P0: s[i,b] = style @ mod_w.T + mod_b                     (PE)
  W-phase per (ib, s): psum = (s_i/k)*wt + sum_j (c_j s_i)*bv_j
    via fp8 DoubleRow pair-matmuls (basis) + bf16 identity matmul (wt);
    ACT evac (scale k) -> W1s bf16 = s*(wt + k*delta_unnorm).
    DVE Winograd weight transform (integer G) -> U[i, uv, o] bf16.
    demod accum: ACT Square(W1s) + PE ones-contraction -> drow.
  C-phase per sample, tile-chunks of 256 (8 tile-rows x 32 tile-cols):
    DVE input transform (col-stage on deinterleaved cols, row-stage) ->
    V bf16; PE matmuls (16 uv x 4 ob x 4 ib, 256-col) accumulating over
    i; ACT evac psum->Mst bf16 with per-class Winograd scale * demod;
    DVE inverse transform (A^T M A) -> bf16 out tiles; DMA out.
  host: reassemble [y,x,tile] layout to [o,h,w], cast f32.
"""

import sys

sys.path.insert(0, "/opt/trn_rl_repo")

import numpy as np
import ml_dtypes
from contextlib import ExitStack

import concourse.bass as bass
import concourse.tile as tile
from concourse import bacc, bass_utils, mybir

F32 = mybir.dt.float32
BF16 = mybir.dt.bfloat16
FP8 = mybir.dt.float8e4
AF = mybir.ActivationFunctionType
ALU = mybir.AluOpType
DR = mybir.MatmulPerfMode.DoubleRow

B, CIN, COUT, K, H, W = 16, 512, 512, 3, 64, 64
STYLE_DIM, BASIS, DIRS = 512, 8, 8
NCORES = 8
BLOC = B // NCORES  # 2 samples per core
NIB = CIN // 128  # 4 i blocks
NOB = COUT // 128  # 4 o blocks
KK = K * K  # 9
NCH = 4  # tile chunks per sample (8 tile-rows each)
NTR = 8  # tile-rows per chunk
NTC = 32  # tile-cols
NPAIR = BASIS // 2
SCALE = 1.0 / np.sqrt(CIN * K * K)
# v (and l) natural index -> class-ordered slot: v in {0,3} -> {0,1}, {1,2} -> {2,3}
VSLOT = {0: 0, 3: 1, 1: 2, 2: 3}

_NC_CACHE = {}
_RUN_KWARGS = {}
_LAST_RESULT = {}


def _build():
    nc = bacc.Bacc("TRN2", target_bir_lowering=False, debug=False)

    # ---- DRAM tensors ----
    xde_d = nc.dram_tensor("xde", [BLOC, NCH, NIB, 128, 18, 66], BF16, kind="ExternalInput")
    styleT_d = nc.dram_tensor("styleT", [STYLE_DIM, BLOC], F32, kind="ExternalInput")
    mod_wT_d = nc.dram_tensor("mod_wT", [STYLE_DIM, CIN], F32, kind="ExternalInput")
    modb_d = nc.dram_tensor("mod_b_t", [128, NIB], F32, kind="ExternalInput")
    wt_d = nc.dram_tensor("wt", [NIB, 128, KK, COUT], BF16, kind="ExternalInput")
    bvp_d = nc.dram_tensor("bvp", [NPAIR, NIB, 128, KK, 2, COUT], FP8, kind="ExternalInput")
    cbc_d = nc.dram_tensor("c_bcast", [128, BLOC * BASIS], F32, kind="ExternalInput")
    kinv_d = nc.dram_tensor("kinv_bcast", [128, BLOC], F32, kind="ExternalInput")
    kt_d = nc.dram_tensor("kt_bcast", [128, BLOC], F32, kind="ExternalInput")
    ident_d = nc.dram_tensor("identity_bf", [128, 128], BF16, kind="ExternalInput")
    ident8_d = nc.dram_tensor("identity_f8", [128, 128], FP8, kind="ExternalInput")
    ones_d = nc.dram_tensor("ones_col", [128, 1], F32, kind="ExternalInput")
    out_d = nc.dram_tensor("out", [BLOC, NOB, NCH, 128, 2, 2, NTR, NTC], BF16, kind="ExternalOutput")
    w1s_d = nc.dram_tensor("w1s_scratch", [NIB, 128, KK, COUT], BF16, kind="Internal")

    with tile.TileContext(nc) as tc, ExitStack() as top:
        persist = top.enter_context(tc.tile_pool(name="persist", bufs=1))

        ident_t = persist.tile([128, 128], BF16, tag="ident")
        nc.sync.dma_start(ident_t[:], ident_d.ap())
        ident8_t = persist.tile([128, 128], FP8, tag="ident8")
        nc.sync.dma_start(ident8_t[:], ident8_d.ap())
        cbc_t = persist.tile([128, BLOC * BASIS], F32, tag="cbc")
        nc.sync.dma_start(cbc_t[:], cbc_d.ap())
        kinv_t = persist.tile([128, BLOC], F32, tag="kinv")
        nc.sync.dma_start(kinv_t[:], kinv_d.ap())
        kt_t = persist.tile([128, BLOC], F32, tag="kt")
        nc.sync.dma_start(kt_t[:], kt_d.ap())
        modb_t = persist.tile([128, NIB], F32, tag="modb")
        nc.sync.dma_start(modb_t[:], modb_d.ap())
        ones_t = persist.tile([128, 1], F32, tag="ones")
        nc.sync.dma_start(ones_t[:], ones_d.ap())
        ones_bf = persist.tile([128, 1], BF16, tag="onesbf")
        nc.vector.tensor_scalar_mul(ones_bf[:], ones_t[:], 1.0)
        s_sb = persist.tile([128, NIB, BLOC], F32, tag="s_sb")
        sk_sb = persist.tile([128, NIB, BLOC], F32, tag="sk_sb")

        # ---- P0: style modulation s[i, b] ----
        with ExitStack() as p0:
            mw_pool = p0.enter_context(tc.tile_pool(name="mw", bufs=NIB))
            st_pool = p0.enter_context(tc.tile_pool(name="st", bufs=1))
            p0_psum = p0.enter_context(tc.tile_pool(name="p0ps", bufs=1, space="PSUM"))
            stT = st_pool.tile([128, NIB, BLOC], F32, tag="styleT")
            nc.sync.dma_start(stT[:], styleT_d.ap().rearrange("(db p) b -> p db b", p=128))
            mw_t = []
            for db in range(NIB):
                t = mw_pool.tile([128, CIN], F32, tag="mw")
                nc.sync.dma_start(t[:], mod_wT_d.ap()[db * 128 : (db + 1) * 128, :])
                mw_t.append(t)
            for ib in range(NIB):
                ps = p0_psum.tile([128, BLOC], F32, tag="ps_s")
                for db in range(NIB):
                    nc.tensor.matmul(
                        ps[:],
                        mw_t[db][:, ib * 128 : (ib + 1) * 128],
                        stT[:, db, :],
                        start=(db == 0),
                        stop=(db == NIB - 1),
                    )
                for s in range(BLOC):
                    nc.vector.tensor_add(
                        s_sb[:, ib, s : s + 1], ps[:, s : s + 1], modb_t[:, ib : ib + 1]
                    )
                    # sk = s_i / k_b
                    nc.vector.tensor_mul(
                        sk_sb[:, ib, s : s + 1],
                        s_sb[:, ib, s : s + 1],
                        kinv_t[:, s : s + 1],
                    )

        # ---- main pools ----
        ci_pool = top.enter_context(tc.tile_pool(name="ci", bufs=12))
        ds_pool = top.enter_context(tc.tile_pool(name="ds", bufs=1))
        bvp_pool = top.enter_context(tc.tile_pool(name="bvp", bufs=5))
        wtc_pool = top.enter_context(tc.tile_pool(name="wtc", bufs=1))
        wts_pool = top.enter_context(tc.tile_pool(name="wts", bufs=1))
        w1s_pool = top.enter_context(tc.tile_pool(name="w1s", bufs=1))
        g_pool = top.enter_context(tc.tile_pool(name="g", bufs=1))
        u_pool = top.enter_context(tc.tile_pool(name="u", bufs=1))
        sq_pool = top.enter_context(tc.tile_pool(name="sq", bufs=1))
        dem_pool = top.enter_context(tc.tile_pool(name="dem", bufs=4))
        drow_pool = top.enter_context(tc.tile_pool(name="drow", bufs=1))
        xde_pool = top.enter_context(tc.tile_pool(name="xde", bufs=1))
        f_pool = top.enter_context(tc.tile_pool(name="f", bufs=1))
        v_pool = top.enter_context(tc.tile_pool(name="v", bufs=16))
        mst_pool = top.enter_context(tc.tile_pool(name="mst", bufs=4))
        sy_pool = top.enter_context(tc.tile_pool(name="sy", bufs=1))
        it_pool = top.enter_context(tc.tile_pool(name="it", bufs=1))
        outt_pool = top.enter_context(tc.tile_pool(name="outt", bufs=1))
        pd_psum = top.enter_context(tc.tile_pool(name="pd", bufs=2, space="PSUM"))
        pc_psum = top.enter_context(tc.tile_pool(name="pc", bufs=2, space="PSUM"))
        psd_psum = top.enter_context(tc.tile_pool(name="psd", bufs=2, space="PSUM"))

        # U tiles: one per ib, rewritten per sample
        u_t = [
            u_pool.tile([128, 16, COUT], BF16, tag=f"u{ib}", name=f"u{ib}")
            for ib in range(NIB)
        ]

        def make_ci(ib, s):
            """ci pair tiles [128, 2, 128] fp8 = diag(s) * c_j for this (ib, s)."""
            ds = ds_pool.tile([128, 128], FP8, tag="ds")
            nc.vector.tensor_scalar_mul(ds[:], ident8_t[:], s_sb[:, ib, s : s + 1])
            cis = []
            for p in range(NPAIR):
                t = ci_pool.tile([128, 2, 128], FP8, tag="ci")
                for half in range(2):
                    j = 2 * p + half
                    nc.vector.tensor_scalar_mul(
                        t[:, half, :], ds[:], cbc_t[:, s * BASIS + j : s * BASIS + j + 1]
                    )
                cis.append(t)
            return cis

        def weight_transform(w1s, ib):
            """DVE Winograd weight transform W1s [128,9,512] -> u_t[ib] [128,16,512]
            with the su*sv class scales folded in (su = [1,.5,.5,1])."""
            U = u_t[ib]
            tmpg = g_pool.tile([128, 3, COUT], BF16, tag="tmpg")
            g1 = g_pool.tile([128, 3, COUT], BF16, tag="g1")
            g2 = g_pool.tile([128, 3, COUT], BF16, tag="g2")
            nc.vector.tensor_add(tmpg[:], w1s[:, 0:3, :], w1s[:, 6:9, :])
            nc.vector.tensor_add(g1[:], tmpg[:], w1s[:, 3:6, :])
            nc.vector.tensor_sub(g2[:], tmpg[:], w1s[:, 3:6, :])
            # su scale for rows u in {1,2}
            nc.vector.tensor_scalar_mul(g1[:], g1[:], 0.5)
            nc.vector.tensor_scalar_mul(g2[:], g2[:], 0.5)
            gu = [w1s[:, 0:3, :], g1[:], g2[:], w1s[:, 6:9, :]]
            for u in range(4):
                gs = gu[u]
                base = u * 4
                nc.vector.tensor_copy(U[:, base + 0, :], gs[:, 0, :])
                nc.vector.tensor_copy(U[:, base + 1, :], gs[:, 2, :])
                tmpc = g_pool.tile([128, COUT], BF16, tag="tmpc")
                nc.vector.tensor_add(tmpc[:], gs[:, 0, :], gs[:, 2, :])
                th = g_pool.tile([128, COUT], BF16, tag="th")
                nc.vector.tensor_scalar_mul(th[:], tmpc[:], 0.5)
                gh = g_pool.tile([128, COUT], BF16, tag="gh")
                nc.vector.tensor_scalar_mul(gh[:], gs[:, 1, :], 0.5)
                nc.vector.tensor_add(U[:, base + 2, :], th[:], gh[:])
                nc.vector.tensor_sub(U[:, base + 3, :], th[:], gh[:])

        def demod_accum(w1s, ib, dsum):
            """ACT square + PE ones-contraction; per-(ib,ob) closed psum group,
            partials accumulated into SBUF row dsum [1, NOB*128]."""
            for ob in range(NOB):
                sq = sq_pool.tile([128, KK, 128], BF16, tag="sq")
                nc.scalar.activation(
                    sq[:], w1s[:, :, ob * 128 : (ob + 1) * 128], AF.Square
                )
                pp = psd_psum.tile([1, 128], F32, tag="psd", name="pp")
                for kk in range(KK):
                    nc.tensor.matmul(
                        pp[:],
                        ones_bf[:],
                        sq[:, kk, :],
                        start=(kk == 0),
                        stop=(kk == KK - 1),
                    )
                dslice = dsum[0:1, ob * 128 : (ob + 1) * 128]
                if ib == 0:
                    nc.vector.tensor_copy(dslice, pp[:])
                else:
                    nc.vector.tensor_add(dslice, dslice, pp[:])

        def demod_finish(s, dsum):
            """drow per ob -> dem_cls [128, 3] tiles (dem*{1,.5,.25})."""
            dem_cls = []
            for ob in range(NOB):
                vv = drow_pool.tile([1, 128], F32, tag="vv")
                nc.vector.tensor_scalar(
                    vv[:], dsum[0:1, ob * 128 : (ob + 1) * 128], SCALE * SCALE, 1e-8,
                    op0=ALU.mult, op1=ALU.add,
                )
                rr = drow_pool.tile([1, 128], F32, tag="rr")
                nc.vector.reciprocal(rr[:], vv[:])
                hh = drow_pool.tile([1, 128], F32, tag="hh")
                nc.scalar.sqrt(hh[:], rr[:])
                drow = drow_pool.tile([1, 128], F32, tag="drow")
                nc.vector.tensor_scalar_mul(drow[:], hh[:], SCALE)
                pst = pd_psum.tile([128, 1], F32, tag="pd", name="pst")
                nc.tensor.matmul(pst[:], drow[:], ones_t[0:1, 0:1])
                dc = dem_pool.tile([128, 1], F32, tag="demc")
                nc.vector.tensor_copy(dc[:], pst[:])
                dem_cls.append(dc)
            return dem_cls

        def delta_w1s_both(ib, cis2):
            """Compute W1s bf16 [128, KK, COUT] for both samples of (ib),
            streaming each basis kk-group once."""
            w1s2 = [
                w1s_pool.tile([128, KK, COUT], BF16, tag=f"w1s{s}", name=f"w1s{s}")
                for s in range(BLOC)
            ]
            for g in range(3):
                bvg = []
                for p in range(NPAIR):
                    t = bvp_pool.tile([128, 3, 2, COUT], FP8, tag="bvp", name=f"bvg{p}")
                    nc.sync.dma_start(t[:], bvp_d.ap()[p, ib, :, 3 * g : 3 * g + 3, :, :])
                    bvg.append(t)
                wtcg = wtc_pool.tile([128, 3, COUT], BF16, tag="wtc")
                nc.sync.dma_start(wtcg[:], wt_d.ap()[ib, :, 3 * g : 3 * g + 3, :])
                for s in range(BLOC):
                    for kl in range(3):
                        kk = 3 * g + kl
                        wts = wts_pool.tile([128, COUT], BF16, tag="wts")
                        nc.scalar.activation(
                            wts[:], wtcg[:, kl, :], AF.Copy,
                            scale=sk_sb[:, ib, s : s + 1],
                        )
                        ps = pd_psum.tile([128, COUT], F32, tag="pd")
                        for p in range(NPAIR):
                            nc.tensor.matmul(
                                ps[:],
                                cis2[s][p][:],
                                bvg[p][:, kl, :, :],
                                start=(p == 0),
                                stop=False,
                                perf_mode=DR,
                            )
                        nc.tensor.matmul(ps[:], ident_t[:], wts[:], start=False, stop=True)
                        nc.scalar.activation(
                            w1s2[s][:, kk, :], ps[:], AF.Copy, scale=kt_t[:, s : s + 1]
                        )
            return w1s2

        # ---- W-phase: sample 0 transforms + sample 1 staged to DRAM ----
        psd_s0 = persist.tile([1, NOB * 128], F32, tag="dsum0", name="dsum0")
        for ib in range(NIB):
            cis2 = [make_ci(ib, s) for s in range(BLOC)]
            w1s2 = delta_w1s_both(ib, cis2)
            # sample 0: full pipeline to U; sample 1: stash to DRAM
            weight_transform(w1s2[0], ib)
            demod_accum(w1s2[0], ib, psd_s0)
            nc.sync.dma_start(w1s_d.ap()[ib], w1s2[1][:])
        dem_cls_s = [demod_finish(0, psd_s0), None]

        # ---- C-phase ----
        def conv_sample(s, dem_cls):
            for ch in range(NCH):
                v_t = {}
                for ib in range(NIB):
                    xs = xde_pool.tile([128, 18, 66], BF16, tag="xde")
                    nc.sync.dma_start(xs[:], xde_d.ap()[s, ch, ib])
                    xv = xs[:].rearrange("p r (two c) -> p r two c", two=2)
                    f4 = f_pool.tile([128, 18, 4, NTC], BF16, tag="f4")
                    E0 = xv[:, :, 0, 0:32]
                    E1 = xv[:, :, 0, 1:33]
                    O0 = xv[:, :, 1, 0:32]
                    O1 = xv[:, :, 1, 1:33]
                    nc.vector.tensor_sub(f4[:, :, 0, :], E0, E1)
                    nc.vector.tensor_add(f4[:, :, 1, :], O0, E1)
                    nc.vector.tensor_sub(f4[:, :, 2, :], E1, O0)
                    nc.vector.tensor_sub(f4[:, :, 3, :], O0, O1)
                    fr = f4[:].rearrange("p (rp two) l c -> p rp two l c", two=2)
                    r1 = fr[:, 0:8, 1, :, :]
                    r2 = fr[:, 1:9, 0, :, :]
                    for k in range(4):
                        vt = v_pool.tile([128, NTR, 4, NTC], BF16, tag="v")
                        if k == 0:
                            nc.vector.tensor_sub(vt[:], fr[:, 0:8, 0, :, :], r2)
                        elif k == 1:
                            nc.vector.tensor_add(vt[:], r1, r2)
                        elif k == 2:
                            nc.vector.tensor_sub(vt[:], r2, r1)
                        else:
                            nc.vector.tensor_sub(vt[:], r1, fr[:, 1:9, 1, :, :])
                        v_t[(ib, k)] = vt
                msts = [
                    mst_pool.tile([128, 4, 4, 256], BF16, tag="mst", name=f"mst{ob}")
                    for ob in range(NOB)
                ]
                for u in range(4):
                    for ob in range(NOB):
                        psq = pc_psum.tile([128, 4, 256], F32, tag="pc")
                        for v in range(4):
                            slot = VSLOT[v]
                            for ib in range(NIB):
                                nc.tensor.matmul(
                                    psq[:, slot, :],
                                    u_t[ib][:, u * 4 + slot, ob * 128 : (ob + 1) * 128],
                                    v_t[(ib, u)][:, :, v, :],
                                    start=(ib == 0),
                                    stop=(ib == NIB - 1),
                                )
                        nc.scalar.activation(
                            msts[ob][:, u, :, :], psq[:], AF.Copy,
                            scale=dem_cls[ob][:],
                        )
                for ob in range(NOB):
                    mst = msts[ob]
                    # inverse transform
                    sy = sy_pool.tile([128, 2, 4, 256], BF16, tag="sy")
                    tmp1 = it_pool.tile([128, 4, 256], BF16, tag="it1")
                    nc.vector.tensor_add(tmp1[:], mst[:, 0, :, :], mst[:, 1, :, :])
                    nc.vector.tensor_add(sy[:, 0, :, :], tmp1[:], mst[:, 2, :, :])
                    tmp2 = it_pool.tile([128, 4, 256], BF16, tag="it2", name="tmp2")
                    nc.vector.tensor_sub(tmp2[:], mst[:, 1, :, :], mst[:, 2, :, :])
                    nc.vector.tensor_sub(sy[:, 1, :, :], tmp2[:], mst[:, 3, :, :])
                    outt = outt_pool.tile([128, 2, 2, NTR, NTC], BF16, tag="outt")
                    l0, l1, l2, l3 = VSLOT[0], VSLOT[1], VSLOT[2], VSLOT[3]
                    ta = it_pool.tile([128, 2, 256], BF16, tag="ita")
                    nc.vector.tensor_add(ta[:], sy[:, :, l1, :], sy[:, :, l2, :])
                    ov0 = outt[:, :, 0, :, :].rearrange("p y r c -> p y (r c)")
                    nc.vector.tensor_add(ov0, ta[:], sy[:, :, l0, :])
                    tb = it_pool.tile([128, 2, 256], BF16, tag="ita", name="tb")
                    nc.vector.tensor_sub(tb[:], sy[:, :, l1, :], sy[:, :, l2, :])
                    ov1 = outt[:, :, 1, :, :].rearrange("p y r c -> p y (r c)")
                    nc.vector.tensor_sub(ov1, tb[:], sy[:, :, l3, :])
                    nc.sync.dma_start(out_d.ap()[s, ob, ch], outt[:])

        conv_sample(0, dem_cls_s[0])

        # ---- sample 1: reload W1s, transform, demod, conv ----
        psd_s1 = persist.tile([1, NOB * 128], F32, tag="dsum1", name="dsum1")
        for ib in range(NIB):
            w1s1 = w1s_pool.tile([128, KK, COUT], BF16, tag="w1s1", name="w1s1r")
            nc.sync.dma_start(w1s1[:], w1s_d.ap()[ib])
            weight_transform(w1s1, ib)
            demod_accum(w1s1, ib, psd_s1)
        dem_cls_s[1] = demod_finish(1, psd_s1)
        conv_sample(1, dem_cls_s[1])

    nc.compile()
    return nc


def _get_nc():
    if "nc" not in _NC_CACHE:
        _NC_CACHE["nc"] = _build()
    return _NC_CACHE["nc"]


def kernel(**inputs):
    x = np.asarray(inputs["x"], dtype=np.float32)
    style = np.asarray(inputs["style"], dtype=np.float32)
    weight = np.asarray(inputs["weight"], dtype=np.float32)
    mod_w = np.asarray(inputs["mod_w"], dtype=np.float32)
    mod_b = np.asarray(inputs["mod_b"], dtype=np.float32)
    bv = np.asarray(inputs["basis_vectors"], dtype=np.float32)
    shifts_coords = np.asarray(inputs["shifts_coords"], dtype=np.float32)
    batch_shifts = np.asarray(inputs["batch_shifts"], dtype=np.float32)
    batch_directions = np.asarray(inputs["batch_directions"])

    # ---- host-side coefficient prep ----
    coefs = shifts_coords[batch_directions].astype(np.float64)  # [B, 8]
    bvf = bv.reshape(BASIS, -1).astype(np.float64)
    G8 = bvf @ bvf.T
    nrm2 = np.einsum("bi,ij,bj->b", coefs, G8, coefs)
    nrm = np.sqrt(np.maximum(nrm2, 0.0))
    k = batch_shifts.astype(np.float64) / np.maximum(nrm, 1e-12)
    k_eff = np.maximum(k, 1e-20)
    kinv = (1.0 / k_eff).astype(np.float32)
    kt = k_eff.astype(np.float32)
    coefs = coefs.astype(np.float32)

    # ---- host-side layout prep (shared across cores) ----
    # weight [1,O,I,3,3] -> [ib, i, kk, o] bf16
    wtl = weight[0].transpose(1, 2, 3, 0).reshape(NIB, 128, KK, COUT)
    wt_bf = np.ascontiguousarray(wtl).astype(ml_dtypes.bfloat16)
    # basis [8,1,O,I,3,3] -> pairs [pair, ib, i, kk, 2, o] fp8
    bvl = bv[:, 0].transpose(0, 2, 3, 4, 1).reshape(BASIS, NIB, 128, KK, COUT)
    bvp = np.ascontiguousarray(
        bvl.reshape(NPAIR, 2, NIB, 128, KK, COUT).transpose(0, 2, 3, 4, 1, 5)
    ).astype(ml_dtypes.float8_e4m3)
    mod_wT = np.ascontiguousarray(mod_w.T)
    mod_b_t = np.ascontiguousarray(mod_b.reshape(NIB, 128).T)
    identity_bf = np.eye(128, dtype=ml_dtypes.bfloat16)
    identity_f8 = np.eye(128, dtype=ml_dtypes.float8_e4m3)
    ones_col = np.ones((128, 1), np.float32)

    # x: pad to 66x66, deinterleave columns, slab by chunk (18 rows, overlap 2)
    xb = x.astype(ml_dtypes.bfloat16)
    xp = np.zeros((B, CIN, H + 2, W + 2), dtype=ml_dtypes.bfloat16)
    xp[:, :, 1 : H + 1, 1 : W + 1] = xb
    # [B, CIN, 66, 2, 33]: col c = 2*cc + parity
    xd = xp.reshape(B, CIN, H + 2, 33, 2).transpose(0, 1, 2, 4, 3)
    # slabs [B, NCH, CIN, 18, 2, 33]
    xs = np.stack([xd[:, :, 16 * ch : 16 * ch + 18] for ch in range(NCH)], axis=1)
    xs = np.ascontiguousarray(xs.reshape(B, NCH, NIB, 128, 18, 66))

    in_maps = []
    for c in range(NCORES):
        sl = slice(c * BLOC, (c + 1) * BLOC)
        in_maps.append(
            {
                "xde": np.ascontiguousarray(xs[sl]),
                "styleT": np.ascontiguousarray(style[sl].T),
                "mod_wT": mod_wT,
                "mod_b_t": mod_b_t,
                "wt": wt_bf,
                "bvp": bvp,
                "c_bcast": np.ascontiguousarray(
                    np.broadcast_to(coefs[sl].reshape(1, -1), (128, BLOC * BASIS))
                ).astype(np.float32),
                "kinv_bcast": np.ascontiguousarray(
                    np.broadcast_to(kinv[sl].reshape(1, -1), (128, BLOC))
                ),
                "kt_bcast": np.ascontiguousarray(
                    np.broadcast_to(kt[sl].reshape(1, -1), (128, BLOC))
                ),
                "identity_bf": identity_bf,
                "identity_f8": identity_f8,
                "ones_col": ones_col,
            }
        )

    nc = _get_nc()
    res = bass_utils.run_bass_kernel_spmd(
        nc, in_maps, core_ids=list(range(NCORES)), **_RUN_KWARGS
    )
    _LAST_RESULT["res"] = res
    # reassemble: out_d [BLOC, NOB, NCH, 128, 2y, 2x, 8tr, 32tc]
    outs = []
    for c in range(NCORES):
        o = np.asarray(res.results[c]["out"]).astype(np.float32)
        # -> [b, ob, o128, ch, tr, y, tc, x] -> [b, 512, 64, 64]
        o = o.transpose(0, 1, 3, 2, 6, 4, 7, 5).reshape(BLOC, COUT, H, W)
        outs.append(o)
    return np.concatenate(outs, axis=0)


# revision 33
# speedup vs baseline: 1.2135x; 1.0532x over previous
"""Trainium2 Bass kernel for DeformableSubspaceModulatedConv2d.

Contract: kernel(**inputs) takes FULL unsharded inputs (as produced by
setup_inputs) and returns the FULL output [16, 512, 64, 64] f32.

Strategy (data-parallel over batch, 2 samples per core on 8 cores),
Winograd F(2x2, 3x3):
  host layout prep: pad+column-deinterleave x (bf16), relayout
    weight/basis; 8x8 basis Gram matrix -> per-sample delta-norm scalar
    k_b = shift / max(||sum_j c_j bv_j||, 1e-12).
  P0: s[i,b] = style @ mod_w.T + mod_b                     (PE)
  W-phase per (ib, s): psum = (s_i/k)*wt + sum_j (c_j s_i)*bv_j
    via fp8 DoubleRow pair-matmuls (basis) + bf16 identity matmul (wt);
    ACT evac (scale k) -> W1s bf16 = s*(wt + k*delta_unnorm).
    DVE Winograd weight transform (integer G) -> U[i, uv, o] bf16.
    demod accum: ACT Square(W1s) + PE ones-contraction -> drow.
  C-phase per sample, tile-chunks of 256 (8 tile-rows x 32 tile-cols):
    DVE input transform (col-stage on deinterleaved cols, row-stage) ->
    V bf16; PE matmuls (16 uv x 4 ob x 4 ib, 256-col) accumulating over
    i; ACT evac psum->Mst bf16 with per-class Winograd scale * demod;
    DVE inverse transform (A^T M A) -> bf16 out tiles; DMA out.
  host: reassemble [y,x,tile] layout to [o,h,w], cast f32.
"""

import sys

sys.path.insert(0, "/opt/trn_rl_repo")

import numpy as np
import ml_dtypes
from contextlib import ExitStack

import concourse.bass as bass
import concourse.tile as tile
from concourse import bacc, bass_utils, mybir

F32 = mybir.dt.float32
BF16 = mybir.dt.bfloat16
FP8 = mybir.dt.float8e4
AF = mybir.ActivationFunctionType
ALU = mybir.AluOpType
DR = mybir.MatmulPerfMode.DoubleRow

B, CIN, COUT, K, H, W = 16, 512, 512, 3, 64, 64
STYLE_DIM, BASIS, DIRS = 512, 8, 8
NCORES = 8
BLOC = B // NCORES  # 2 samples per core
NIB = CIN // 128  # 4 i blocks
NOB = COUT // 128  # 4 o blocks
KK = K * K  # 9
NCH = 4  # tile chunks per sample (8 tile-rows each)
NTR = 8  # tile-rows per chunk
NTC = 32  # tile-cols
NPAIR = BASIS // 2
SCALE = 1.0 / np.sqrt(CIN * K * K)
# v (and l) natural index -> class-ordered slot: v in {0,3} -> {0,1}, {1,2} -> {2,3}
VSLOT = {0: 0, 3: 1, 1: 2, 2: 3}

_NC_CACHE = {}
_RUN_KWARGS = {}
_LAST_RESULT = {}


def _build():
    nc = bacc.Bacc("TRN2", target_bir_lowering=False, debug=False)

    # ---- DRAM tensors ----
    xde_d = nc.dram_tensor("xde", [BLOC, NCH, NIB, 128, 18, 66], BF16, kind="ExternalInput")
    styleT_d = nc.dram_tensor("styleT", [STYLE_DIM, BLOC], F32, kind="ExternalInput")
    mod_wT_d = nc.dram_tensor("mod_wT", [STYLE_DIM, CIN], F32, kind="ExternalInput")
    modb_d = nc.dram_tensor("mod_b_t", [128, NIB], F32, kind="ExternalInput")
    wt_d = nc.dram_tensor("wt", [NIB, 128, KK, COUT], BF16, kind="ExternalInput")
    bvp_d = nc.dram_tensor("bvp", [NPAIR, NIB, 128, KK, 2, COUT], FP8, kind="ExternalInput")
    cbc_d = nc.dram_tensor("c_bcast", [128, BLOC * BASIS], F32, kind="ExternalInput")
    kinv_d = nc.dram_tensor("kinv_bcast", [128, BLOC], F32, kind="ExternalInput")
    kt_d = nc.dram_tensor("kt_bcast", [128, BLOC], F32, kind="ExternalInput")
    ident_d = nc.dram_tensor("identity_bf", [128, 128], BF16, kind="ExternalInput")
    ident8_d = nc.dram_tensor("identity_f8", [128, 128], FP8, kind="ExternalInput")
    ones_d = nc.dram_tensor("ones_col", [128, 1], F32, kind="ExternalInput")
    out_d = nc.dram_tensor("out", [BLOC, NOB, NCH, 128, 2, 2, NTR, NTC], BF16, kind="ExternalOutput")
    w1s_d = nc.dram_tensor("w1s_scratch", [NIB, 128, KK, COUT], BF16, kind="Internal")

    with tile.TileContext(nc) as tc, ExitStack() as top:
        persist = top.enter_context(tc.tile_pool(name="persist", bufs=1))

        ident_t = persist.tile([128, 128], BF16, tag="ident")
        nc.sync.dma_start(ident_t[:], ident_d.ap())
        ident8_t = persist.tile([128, 128], FP8, tag="ident8")
        nc.sync.dma_start(ident8_t[:], ident8_d.ap())
        cbc_t = persist.tile([128, BLOC * BASIS], F32, tag="cbc")
        nc.sync.dma_start(cbc_t[:], cbc_d.ap())
        kinv_t = persist.tile([128, BLOC], F32, tag="kinv")
        nc.sync.dma_start(kinv_t[:], kinv_d.ap())
        kt_t = persist.tile([128, BLOC], F32, tag="kt")
        nc.sync.dma_start(kt_t[:], kt_d.ap())
        modb_t = persist.tile([128, NIB], F32, tag="modb")
        nc.sync.dma_start(modb_t[:], modb_d.ap())
        ones_t = persist.tile([128, 1], F32, tag="ones")
        nc.sync.dma_start(ones_t[:], ones_d.ap())
        ones_bf = persist.tile([128, 1], BF16, tag="onesbf")
        nc.vector.tensor_scalar_mul(ones_bf[:], ones_t[:], 1.0)
        s_sb = persist.tile([128, NIB, BLOC], F32, tag="s_sb")
        sk_sb = persist.tile([128, NIB, BLOC], F32, tag="sk_sb")

        # ---- P0: style modulation s[i, b] ----
        with ExitStack() as p0:
            mw_pool = p0.enter_context(tc.tile_pool(name="mw", bufs=NIB))
            st_pool = p0.enter_context(tc.tile_pool(name="st", bufs=1))
            p0_psum = p0.enter_context(tc.tile_pool(name="p0ps", bufs=1, space="PSUM"))
            stT = st_pool.tile([128, NIB, BLOC], F32, tag="styleT")
            nc.sync.dma_start(stT[:], styleT_d.ap().rearrange("(db p) b -> p db b", p=128))
            mw_t = []
            for db in range(NIB):
                t = mw_pool.tile([128, CIN], F32, tag="mw")
                nc.sync.dma_start(t[:], mod_wT_d.ap()[db * 128 : (db + 1) * 128, :])
                mw_t.append(t)
            for ib in range(NIB):
                ps = p0_psum.tile([128, BLOC], F32, tag="ps_s")
                for db in range(NIB):
                    nc.tensor.matmul(
                        ps[:],
                        mw_t[db][:, ib * 128 : (ib + 1) * 128],
                        stT[:, db, :],
                        start=(db == 0),
                        stop=(db == NIB - 1),
                    )
                for s in range(BLOC):
                    nc.vector.tensor_add(
                        s_sb[:, ib, s : s + 1], ps[:, s : s + 1], modb_t[:, ib : ib + 1]
                    )
                    # sk = s_i / k_b
                    nc.vector.tensor_mul(
                        sk_sb[:, ib, s : s + 1],
                        s_sb[:, ib, s : s + 1],
                        kinv_t[:, s : s + 1],
                    )

        # ---- main pools ----
        ci_pool = top.enter_context(tc.tile_pool(name="ci", bufs=12))
        ds_pool = top.enter_context(tc.tile_pool(name="ds", bufs=1))
        bvp_pool = top.enter_context(tc.tile_pool(name="bvp", bufs=5))
        wtc_pool = top.enter_context(tc.tile_pool(name="wtc", bufs=1))
        wts_pool = top.enter_context(tc.tile_pool(name="wts", bufs=1))
        w1s_pool = top.enter_context(tc.tile_pool(name="w1s", bufs=1))
        g_pool = top.enter_context(tc.tile_pool(name="g", bufs=1))
        u_pool = top.enter_context(tc.tile_pool(name="u", bufs=1))
        sq_pool = top.enter_context(tc.tile_pool(name="sq", bufs=1))
        dem_pool = top.enter_context(tc.tile_pool(name="dem", bufs=4))
        drow_pool = top.enter_context(tc.tile_pool(name="drow", bufs=1))
        xde_pool = top.enter_context(tc.tile_pool(name="xde", bufs=1))
        f_pool = top.enter_context(tc.tile_pool(name="f", bufs=1))
        v_pool = top.enter_context(tc.tile_pool(name="v", bufs=16))
        mst_pool = top.enter_context(tc.tile_pool(name="mst", bufs=4))
        sy_pool = top.enter_context(tc.tile_pool(name="sy", bufs=1))
        it_pool = top.enter_context(tc.tile_pool(name="it", bufs=1))
        outt_pool = top.enter_context(tc.tile_pool(name="outt", bufs=1))
        pd_psum = top.enter_context(tc.tile_pool(name="pd", bufs=2, space="PSUM"))
        pc_psum = top.enter_context(tc.tile_pool(name="pc", bufs=2, space="PSUM"))
        psd_psum = top.enter_context(tc.tile_pool(name="psd", bufs=2, space="PSUM"))

        # U tiles: one per ib, rewritten per sample
        u_t = [
            u_pool.tile([128, 16, COUT], BF16, tag=f"u{ib}", name=f"u{ib}")
            for ib in range(NIB)
        ]

        def make_ci(ib, s):
            """ci pair tiles [128, 2, 128] fp8 = diag(s) * c_j for this (ib, s)."""
            ds = ds_pool.tile([128, 128], FP8, tag="ds")
            nc.vector.tensor_scalar_mul(ds[:], ident8_t[:], s_sb[:, ib, s : s + 1])
            cis = []
            for p in range(NPAIR):
                t = ci_pool.tile([128, 2, 128], FP8, tag="ci")
                for half in range(2):
                    j = 2 * p + half
                    nc.vector.tensor_scalar_mul(
                        t[:, half, :], ds[:], cbc_t[:, s * BASIS + j : s * BASIS + j + 1]
                    )
                cis.append(t)
            return cis

        def weight_transform(w1s, ib):
            """DVE Winograd weight transform W1s [128,9,512] -> u_t[ib] [128,16,512]
            with the su*sv class scales folded in (su = [1,.5,.5,1])."""
            U = u_t[ib]
            tmpg = g_pool.tile([128, 3, COUT], BF16, tag="tmpg")
            g1 = g_pool.tile([128, 3, COUT], BF16, tag="g1")
            g2 = g_pool.tile([128, 3, COUT], BF16, tag="g2")
            nc.vector.tensor_add(tmpg[:], w1s[:, 0:3, :], w1s[:, 6:9, :])
            nc.vector.tensor_add(g1[:], tmpg[:], w1s[:, 3:6, :])
            nc.vector.tensor_sub(g2[:], tmpg[:], w1s[:, 3:6, :])
            # su scale for rows u in {1,2}
            nc.vector.tensor_scalar_mul(g1[:], g1[:], 0.5)
            nc.vector.tensor_scalar_mul(g2[:], g2[:], 0.5)
            gu = [w1s[:, 0:3, :], g1[:], g2[:], w1s[:, 6:9, :]]
            for u in range(4):
                gs = gu[u]
                base = u * 4
                nc.vector.tensor_copy(U[:, base + 0, :], gs[:, 0, :])
                nc.vector.tensor_copy(U[:, base + 1, :], gs[:, 2, :])
                tmpc = g_pool.tile([128, COUT], BF16, tag="tmpc")
                nc.vector.tensor_add(tmpc[:], gs[:, 0, :], gs[:, 2, :])
                th = g_pool.tile([128, COUT], BF16, tag="th")
                nc.vector.tensor_scalar_mul(th[:], tmpc[:], 0.5)
                gh = g_pool.tile([128, COUT], BF16, tag="gh")
                nc.vector.tensor_scalar_mul(gh[:], gs[:, 1, :], 0.5)
                nc.vector.tensor_add(U[:, base + 2, :], th[:], gh[:])
                nc.vector.tensor_sub(U[:, base + 3, :], th[:], gh[:])

        def demod_accum(w1s, ib, dsum):
            """ACT square + PE ones-contraction; per-(ib,ob) closed psum group,
            partials accumulated into SBUF row dsum [1, NOB*128]."""
            for ob in range(NOB):
                sq = sq_pool.tile([128, KK, 128], BF16, tag="sq")
                nc.scalar.activation(
                    sq[:], w1s[:, :, ob * 128 : (ob + 1) * 128], AF.Square
                )
                pp = psd_psum.tile([1, 3 * 128], F32, tag="psd", name="pp")
                sqv = sq[:].rearrange("p (g kl) o -> p g (kl o)", g=3)
                for g in range(3):
                    nc.tensor.matmul(
                        pp[:],
                        ones_bf[:],
                        sqv[:, g, :],
                        start=(g == 0),
                        stop=(g == 2),
                    )
                t12 = drow_pool.tile([1, 128], F32, tag="vv", name="t12")
                nc.vector.tensor_copy(t12[:], pp[0:1, 0:128])
                t123 = drow_pool.tile([1, 128], F32, tag="rr", name="t123")
                nc.vector.tensor_add(t123[:], t12[:], pp[0:1, 128:256])
                dslice = dsum[0:1, ob * 128 : (ob + 1) * 128]
                if ib == 0:
                    nc.vector.tensor_add(dslice, t123[:], pp[0:1, 256:384])
                else:
                    nc.vector.tensor_add(t12[:], t123[:], pp[0:1, 256:384])
                    nc.vector.tensor_add(dslice, dslice, t12[:])

        def demod_finish(s, dsum):
            """drow per ob -> dem_cls [128, 3] tiles (dem*{1,.5,.25})."""
            dem_cls = []
            for ob in range(NOB):
                vv = drow_pool.tile([1, 128], F32, tag="vv")
                nc.vector.tensor_scalar(
                    vv[:], dsum[0:1, ob * 128 : (ob + 1) * 128], SCALE * SCALE, 1e-8,
                    op0=ALU.mult, op1=ALU.add,
                )
                rr = drow_pool.tile([1, 128], F32, tag="rr")
                nc.vector.reciprocal(rr[:], vv[:])
                hh = drow_pool.tile([1, 128], F32, tag="hh")
                nc.scalar.sqrt(hh[:], rr[:])
                drow = drow_pool.tile([1, 128], F32, tag="drow")
                nc.vector.tensor_scalar_mul(drow[:], hh[:], SCALE)
                pst = pd_psum.tile([128, 1], F32, tag="pd", name="pst")
                nc.tensor.matmul(pst[:], drow[:], ones_t[0:1, 0:1])
                dc = dem_pool.tile([128, 1], F32, tag="demc")
                nc.vector.tensor_copy(dc[:], pst[:])
                dem_cls.append(dc)
            return dem_cls

        def delta_w1s_both(ib, cis2):
            """Compute W1s bf16 [128, KK, COUT] for both samples of (ib),
            streaming each basis kk-group once."""
            w1s2 = [
                w1s_pool.tile([128, KK, COUT], BF16, tag=f"w1s{s}", name=f"w1s{s}")
                for s in range(BLOC)
            ]
            for g in range(3):
                bvg = []
                for p in range(NPAIR):
                    t = bvp_pool.tile([128, 3, 2, COUT], FP8, tag="bvp", name=f"bvg{p}")
                    nc.sync.dma_start(t[:], bvp_d.ap()[p, ib, :, 3 * g : 3 * g + 3, :, :])
                    bvg.append(t)
                wtcg = wtc_pool.tile([128, 3, COUT], BF16, tag="wtc")
                nc.sync.dma_start(wtcg[:], wt_d.ap()[ib, :, 3 * g : 3 * g + 3, :])
                for s in range(BLOC):
                    wts = wts_pool.tile([128, 3, COUT], BF16, tag="wts")
                    nc.vector.tensor_scalar_mul(
                        wts[:], wtcg[:], sk_sb[:, ib, s : s + 1]
                    )
                    for kl in range(3):
                        kk = 3 * g + kl
                        ps = pd_psum.tile([128, COUT], F32, tag="pd")
                        for p in range(NPAIR):
                            nc.tensor.matmul(
                                ps[:],
                                cis2[s][p][:],
                                bvg[p][:, kl, :, :],
                                start=(p == 0),
                                stop=False,
                                perf_mode=DR,
                            )
                        nc.tensor.matmul(
                            ps[:], ident_t[:], wts[:, kl, :], start=False, stop=True
                        )
                        nc.scalar.activation(
                            w1s2[s][:, kk, :], ps[:], AF.Copy, scale=kt_t[:, s : s + 1]
                        )
            return w1s2

        # ---- W-phase: sample 0 transforms + sample 1 staged to DRAM ----
        psd_s0 = persist.tile([1, NOB * 128], F32, tag="dsum0", name="dsum0")
        for ib in range(NIB):
            cis2 = [make_ci(ib, s) for s in range(BLOC)]
            w1s2 = delta_w1s_both(ib, cis2)
            # sample 0: full pipeline to U; sample 1: stash to DRAM
            weight_transform(w1s2[0], ib)
            demod_accum(w1s2[0], ib, psd_s0)
            nc.sync.dma_start(w1s_d.ap()[ib], w1s2[1][:])
        dem_cls_s = [demod_finish(0, psd_s0), None]

        # ---- C-phase ----
        def conv_sample(s, dem_cls):
            for ch in range(NCH):
                v_t = {}
                for ib in range(NIB):
                    xs = xde_pool.tile([128, 18, 66], BF16, tag="xde")
                    nc.sync.dma_start(xs[:], xde_d.ap()[s, ch, ib])
                    xv = xs[:].rearrange("p r (two c) -> p r two c", two=2)
                    f4 = f_pool.tile([128, 18, 4, NTC], BF16, tag="f4")
                    E0 = xv[:, :, 0, 0:32]
                    E1 = xv[:, :, 0, 1:33]
                    O0 = xv[:, :, 1, 0:32]
                    O1 = xv[:, :, 1, 1:33]
                    nc.vector.tensor_sub(f4[:, :, 0, :], E0, E1)
                    nc.vector.tensor_add(f4[:, :, 1, :], O0, E1)
                    nc.vector.tensor_sub(f4[:, :, 2, :], E1, O0)
                    nc.vector.tensor_sub(f4[:, :, 3, :], O0, O1)
                    fr = f4[:].rearrange("p (rp two) l c -> p rp two l c", two=2)
                    r1 = fr[:, 0:8, 1, :, :]
                    r2 = fr[:, 1:9, 0, :, :]
                    for k in range(4):
                        vt = v_pool.tile([128, NTR, 4, NTC], BF16, tag="v")
                        if k == 0:
                            nc.vector.tensor_sub(vt[:], fr[:, 0:8, 0, :, :], r2)
                        elif k == 1:
                            nc.vector.tensor_add(vt[:], r1, r2)
                        elif k == 2:
                            nc.vector.tensor_sub(vt[:], r2, r1)
                        else:
                            nc.vector.tensor_sub(vt[:], r1, fr[:, 1:9, 1, :, :])
                        v_t[(ib, k)] = vt
                msts = [
                    mst_pool.tile([128, 4, 4, 256], BF16, tag="mst", name=f"mst{ob}")
                    for ob in range(NOB)
                ]
                for u in range(4):
                    for ob in range(NOB):
                        psq = pc_psum.tile([128, 4, 256], F32, tag="pc")
                        for vpair in ((0, 1), (3, 2)):
                            for ib in range(NIB):
                                for v in vpair:
                                    slot = VSLOT[v]
                                    nc.tensor.matmul(
                                        psq[:, slot, :],
                                        u_t[ib][:, u * 4 + slot, ob * 128 : (ob + 1) * 128],
                                        v_t[(ib, u)][:, :, v, :],
                                        start=(ib == 0),
                                        stop=(ib == NIB - 1),
                                    )
                        nc.scalar.activation(
                            msts[ob][:, u, :, :], psq[:], AF.Copy,
                            scale=dem_cls[ob][:],
                        )
                for ob in range(NOB):
                    mst = msts[ob]
                    # inverse transform
                    sy = sy_pool.tile([128, 2, 4, 256], BF16, tag="sy")
                    tmp1 = it_pool.tile([128, 4, 256], BF16, tag="it1")
                    nc.vector.tensor_add(tmp1[:], mst[:, 0, :, :], mst[:, 1, :, :])
                    nc.vector.tensor_add(sy[:, 0, :, :], tmp1[:], mst[:, 2, :, :])
                    tmp2 = it_pool.tile([128, 4, 256], BF16, tag="it1", name="tmp2")
                    nc.vector.tensor_sub(tmp2[:], mst[:, 1, :, :], mst[:, 2, :, :])
                    nc.vector.tensor_sub(sy[:, 1, :, :], tmp2[:], mst[:, 3, :, :])
                    outt = outt_pool.tile([128, 2, 2, NTR, NTC], BF16, tag="outt")
                    l0, l1, l2, l3 = VSLOT[0], VSLOT[1], VSLOT[2], VSLOT[3]
                    ta = it_pool.tile([128, 2, 256], BF16, tag="ita")
                    nc.vector.tensor_add(ta[:], sy[:, :, l1, :], sy[:, :, l2, :])
                    ov0 = outt[:, :, 0, :, :].rearrange("p y r c -> p y (r c)")
                    nc.vector.tensor_add(ov0, ta[:], sy[:, :, l0, :])
                    tb = it_pool.tile([128, 2, 256], BF16, tag="ita", name="tb")
                    nc.vector.tensor_sub(tb[:], sy[:, :, l1, :], sy[:, :, l2, :])
                    ov1 = outt[:, :, 1, :, :].rearrange("p y r c -> p y (r c)")
                    nc.vector.tensor_sub(ov1, tb[:], sy[:, :, l3, :])
                    nc.sync.dma_start(out_d.ap()[s, ob, ch], outt[:])

        conv_sample(0, dem_cls_s[0])

        # ---- sample 1: reload W1s, transform, demod, conv ----
        psd_s1 = persist.tile([1, NOB * 128], F32, tag="dsum1", name="dsum1")
        for ib in range(NIB):
            w1s1 = w1s_pool.tile([128, KK, COUT], BF16, tag="w1s1", name="w1s1r")
            nc.sync.dma_start(w1s1[:], w1s_d.ap()[ib])
            weight_transform(w1s1, ib)
            demod_accum(w1s1, ib, psd_s1)
        dem_cls_s[1] = demod_finish(1, psd_s1)
        conv_sample(1, dem_cls_s[1])

    nc.compile()
    return nc


def _get_nc():
    if "nc" not in _NC_CACHE:
        _NC_CACHE["nc"] = _build()
    return _NC_CACHE["nc"]


def kernel(**inputs):
    x = np.asarray(inputs["x"], dtype=np.float32)
    style = np.asarray(inputs["style"], dtype=np.float32)
    weight = np.asarray(inputs["weight"], dtype=np.float32)
    mod_w = np.asarray(inputs["mod_w"], dtype=np.float32)
    mod_b = np.asarray(inputs["mod_b"], dtype=np.float32)
    bv = np.asarray(inputs["basis_vectors"], dtype=np.float32)
    shifts_coords = np.asarray(inputs["shifts_coords"], dtype=np.float32)
    batch_shifts = np.asarray(inputs["batch_shifts"], dtype=np.float32)
    batch_directions = np.asarray(inputs["batch_directions"])

    # ---- host-side coefficient prep ----
    coefs = shifts_coords[batch_directions].astype(np.float64)  # [B, 8]
    bvf = bv.reshape(BASIS, -1).astype(np.float64)
    G8 = bvf @ bvf.T
    nrm2 = np.einsum("bi,ij,bj->b", coefs, G8, coefs)
    nrm = np.sqrt(np.maximum(nrm2, 0.0))
    k = batch_shifts.astype(np.float64) / np.maximum(nrm, 1e-12)
    k_eff = np.maximum(k, 1e-20)
    kinv = (1.0 / k_eff).astype(np.float32)
    kt = k_eff.astype(np.float32)
    coefs = coefs.astype(np.float32)

    # ---- host-side layout prep (shared across cores) ----
    # weight [1,O,I,3,3] -> [ib, i, kk, o] bf16
    wtl = weight[0].transpose(1, 2, 3, 0).reshape(NIB, 128, KK, COUT)
    wt_bf = np.ascontiguousarray(wtl).astype(ml_dtypes.bfloat16)
    # basis [8,1,O,I,3,3] -> pairs [pair, ib, i, kk, 2, o] fp8
    bvl = bv[:, 0].transpose(0, 2, 3, 4, 1).reshape(BASIS, NIB, 128, KK, COUT)
    bvp = np.ascontiguousarray(
        bvl.reshape(NPAIR, 2, NIB, 128, KK, COUT).transpose(0, 2, 3, 4, 1, 5)
    ).astype(ml_dtypes.float8_e4m3)
    mod_wT = np.ascontiguousarray(mod_w.T)
    mod_b_t = np.ascontiguousarray(mod_b.reshape(NIB, 128).T)
    identity_bf = np.eye(128, dtype=ml_dtypes.bfloat16)
    identity_f8 = np.eye(128, dtype=ml_dtypes.float8_e4m3)
    ones_col = np.ones((128, 1), np.float32)

    # x: pad to 66x66, deinterleave columns, slab by chunk (18 rows, overlap 2)
    xb = x.astype(ml_dtypes.bfloat16)
    xp = np.zeros((B, CIN, H + 2, W + 2), dtype=ml_dtypes.bfloat16)
    xp[:, :, 1 : H + 1, 1 : W + 1] = xb
    # [B, CIN, 66, 2, 33]: col c = 2*cc + parity
    xd = xp.reshape(B, CIN, H + 2, 33, 2).transpose(0, 1, 2, 4, 3)
    # slabs [B, NCH, CIN, 18, 2, 33]
    xs = np.stack([xd[:, :, 16 * ch : 16 * ch + 18] for ch in range(NCH)], axis=1)
    xs = np.ascontiguousarray(xs.reshape(B, NCH, NIB, 128, 18, 66))

    in_maps = []
    for c in range(NCORES):
        sl = slice(c * BLOC, (c + 1) * BLOC)
        in_maps.append(
            {
                "xde": np.ascontiguousarray(xs[sl]),
                "styleT": np.ascontiguousarray(style[sl].T),
                "mod_wT": mod_wT,
                "mod_b_t": mod_b_t,
                "wt": wt_bf,
                "bvp": bvp,
                "c_bcast": np.ascontiguousarray(
                    np.broadcast_to(coefs[sl].reshape(1, -1), (128, BLOC * BASIS))
                ).astype(np.float32),
                "kinv_bcast": np.ascontiguousarray(
                    np.broadcast_to(kinv[sl].reshape(1, -1), (128, BLOC))
                ),
                "kt_bcast": np.ascontiguousarray(
                    np.broadcast_to(kt[sl].reshape(1, -1), (128, BLOC))
                ),
                "identity_bf": identity_bf,
                "identity_f8": identity_f8,
                "ones_col": ones_col,
            }
        )

    nc = _get_nc()
    res = bass_utils.run_bass_kernel_spmd(
        nc, in_maps, core_ids=list(range(NCORES)), **_RUN_KWARGS
    )
    _LAST_RESULT["res"] = res
    # reassemble: out_d [BLOC, NOB, NCH, 128, 2y, 2x, 8tr, 32tc]
    outs = []
    for c in range(NCORES):
        o = np.asarray(res.results[c]["out"]).astype(np.float32)
        # -> [b, ob, o128, ch, tr, y, tc, x] -> [b, 512, 64, 64]
        o = o.transpose(0, 1, 3, 2, 6, 4, 7, 5).reshape(BLOC, COUT, H, W)
        outs.append(o)
    return np.concatenate(outs, axis=0)
